# revision 2
# baseline (speedup 1.0000x reference)
"""LocalGNO Trainium2 kernel: data-parallel over batch (8 cores, 1 batch elem each).

Feature-major layout [H=96 partitions, N=16384 tokens]. Message pass factorized:
  pre_d[t] = A[t] + B[t+d],  A = W1a^T h - w1c (X) coord (+b1 in ACT bias),
                             B = W1b^T h + w1c (X) coord
computed per (tile, shift) as two accumulating PE matmuls (K=97, incl. coord row).
Sum over the 8 shifts folds into 8 accumulating W2^T matmuls in PSUM.
LayerNorm along features via ones-matmul column sums + outer-product broadcasts.
"""
import sys

sys.path.insert(0, "/opt/trn_rl_repo")

import numpy as np

B, N, IN_DIM, HID, K, L = 8, 16384, 4, 96, 4, 2
LN_EPS = 1e-5
T = 512
NT = N // T
PAD = 4
NP = N + 2 * PAD
SHIFTS = [-4, -3, -2, -1, 1, 2, 3, 4]
PAIRS = [(-4, -3), (-2, -1), (1, 2), (3, 4)]

_CACHE = {}


def _install_tile_drain_patch():
    """walrus CoreV3 rejects the Tile kernel-tail Drain when it carries >2 sem
    waits; emit the extras as standalone sync-engine waits instead."""
    import concourse.tile as tile
    from concourse.vector_clock import ScopedClock

    if getattr(tile.TileContext, "_drain_patch", False):
        return

    def _drain_and_barrier_split(self, tick_clock, wait_clock):
        nc = self.nc
        drain_inst = nc.sync.drain()
        wait_clock.add_sem_waits(
            drain_inst.ins, ScopedClock({None: tick_clock.global_clock})
        )
        si = drain_inst.ins.sync_info
        waits = list(si.on_wait) if si is not None and si.on_wait else []
        if len(waits) > 1:
            si.on_wait = waits[:1]
            id2sem = {s.num: s for s in self.sems.allocated().values()}
            for w in waits[1:]:
                nc.sync.wait_ge(id2sem[w.id], w.wait_value)
        nc.all_engine_barrier()
        popped = nc._tile_sem_poison_stack.pop()
        assert popped is self._sem_poison
        nc.clear_and_free_semaphores(list(self.sems.allocated().values()))
        nc.all_engine_barrier()

    tile.TileContext._drain_and_barrier = _drain_and_barrier_split
    tile.TileContext._drain_patch = True


def _split_excess_waits(nc, mybir, maxw=1):
    """walrus CoreV3 rejects instructions carrying more than ~2 sem waits;
    move the overflow onto NoOps inserted just before the offender."""
    f = nc.m.functions[0]
    for bb in f.blocks:
        insts = list(bb.instructions)
        out, changed, k = [], False, 0
        for inst in insts:
            si = inst.sync_info
            waits = list(si.on_wait) if si is not None and si.on_wait else []
            if len(waits) > maxw:
                keep, extras = waits[:maxw], waits[maxw:]
                while extras:
                    chunk, extras = extras[:maxw], extras[maxw:]
                    nop = mybir.InstNoOp(name=f"zzwait{k}_{inst.name}")
                    k += 1
                    nop.engine = inst.engine
                    nop.sync_info = mybir.SyncInfo(on_wait=chunk, on_update=[])
                    out.append(nop)
                si.on_wait = keep
                changed = True
            out.append(inst)
        if changed:
            bb.instructions = out


def _build():
    import concourse.bass as bass
    import concourse.mybir as mybir
    import concourse.tile as tile

    _install_tile_drain_patch()
    f32 = mybir.dt.float32
    AF = mybir.ActivationFunctionType
    ALU = mybir.AluOpType

    nc = bass.Bass()

    def din(name, shape):
        return nc.dram_tensor(name, shape, f32, kind="ExternalInput").ap()

    d_xT = din("xT", [IN_DIM, N])
    d_coordp = din("coordp", [1, NP])
    d_wa = [din(f"wa{l}", [HID + 1, HID]) for l in range(L)]
    d_wb = [din(f"wb{l}", [HID + 1, HID]) for l in range(L)]
    d_w2 = [din(f"w2{l}", [HID, HID]) for l in range(L)]
    d_u1a = [din(f"u1a{l}", [HID, HID]) for l in range(L)]
    d_u1b = [din(f"u1b{l}", [HID, HID]) for l in range(L)]
    d_u2 = [din(f"u2{l}", [HID, HID]) for l in range(L)]
    d_b1 = [din(f"b1{l}", [HID, 1]) for l in range(L)]
    d_bupd = [din(f"bupd{l}", [HID, 1]) for l in range(L)]
    d_b2u = [din(f"b2u{l}", [HID, 1]) for l in range(L)]
    d_ew1 = din("ew1", [IN_DIM, HID])
    d_eb1 = din("eb1", [HID, 1])
    d_ew2 = din("ew2", [HID, HID])
    d_eb2 = din("eb2", [HID, 1])
    d_dw1 = din("dw1", [HID, HID])
    d_db1 = din("db1", [HID, 1])
    d_dw2 = din("dw2", [HID, 1])
    d_db2 = din("db2", [1, 1])
    d_ones96 = din("ones96", [HID, 1])
    d_onesr = din("onesr", [1, HID])
    d_invL = din("invL", [HID, PAD])
    d_invR = din("invR", [HID, PAD])

    d_out = nc.dram_tensor("out", [1, N], f32, kind="ExternalOutput").ap()
    # scratch DRAM for LN stat repacking
    d_s1 = nc.dram_tensor("s1sc", [1, N], f32).ap()
    d_s2 = nc.dram_tensor("s2sc", [1, N], f32).ap()
    d_is = nc.dram_tensor("issc", [1, N], f32).ap()
    d_muis = nc.dram_tensor("muissc", [1, N], f32).ap()

    # persistent SBUF
    hc = nc.alloc_sbuf_tensor("hc", [HID + 1, NP], f32).ap()
    zbuf = nc.alloc_sbuf_tensor("zbuf", [HID, N], f32).ap()

    def sb(name, shape):
        return nc.alloc_sbuf_tensor(name + "_s", shape, f32).ap()

    s_wa = [sb(f"wa{l}", [HID + 1, HID]) for l in range(L)]
    s_wb = [sb(f"wb{l}", [HID + 1, HID]) for l in range(L)]
    s_w2 = [sb(f"w2{l}", [HID, HID]) for l in range(L)]
    s_u1a = [sb(f"u1a{l}", [HID, HID]) for l in range(L)]
    s_u1b = [sb(f"u1b{l}", [HID, HID]) for l in range(L)]
    s_u2 = [sb(f"u2{l}", [HID, HID]) for l in range(L)]
    s_b1 = [sb(f"b1{l}", [HID, 1]) for l in range(L)]
    s_bupd = [sb(f"bupd{l}", [HID, 1]) for l in range(L)]
    s_b2u = [sb(f"b2u{l}", [HID, 1]) for l in range(L)]
    s_ew1 = sb("ew1", [IN_DIM, HID])
    s_eb1 = sb("eb1", [HID, 1])
    s_ew2 = sb("ew2", [HID, HID])
    s_eb2 = sb("eb2", [HID, 1])
    s_dw1 = sb("dw1", [HID, HID])
    s_db1 = sb("db1", [HID, 1])
    s_dw2 = sb("dw2", [HID, 1])
    s_db2 = sb("db2", [1, 1])
    s_ones96 = sb("ones96", [HID, 1])
    s_onesr = sb("onesr", [1, HID])
    s_invL = sb("invL", [HID, PAD])
    s_invR = sb("invR", [HID, PAD])

    MM = nc.tensor.matmul
    H = HID

    with tile.TileContext(nc) as tc:
        # ---- load weights & coord, clear pads
        for s, d in [
            (s_ew1, d_ew1), (s_eb1, d_eb1), (s_ew2, d_ew2), (s_eb2, d_eb2),
            (s_dw1, d_dw1), (s_db1, d_db1), (s_dw2, d_dw2), (s_db2, d_db2),
            (s_ones96, d_ones96), (s_onesr, d_onesr),
            (s_invL, d_invL), (s_invR, d_invR),
        ] + [
            p for l in range(L) for p in [
                (s_wa[l], d_wa[l]), (s_wb[l], d_wb[l]), (s_w2[l], d_w2[l]),
                (s_u1a[l], d_u1a[l]), (s_u1b[l], d_u1b[l]), (s_u2[l], d_u2[l]),
                (s_b1[l], d_b1[l]), (s_bupd[l], d_bupd[l]), (s_b2u[l], d_b2u[l]),
            ]
        ]:
            nc.sync.dma_start(s, d)
        nc.sync.dma_start(hc[H : H + 1, :], d_coordp)
        nc.gpsimd.memset(hc[0:H, 0:PAD], 0.0)
        nc.gpsimd.memset(hc[0:H, N + PAD : NP], 0.0)

        # ---- embed: h = silu(x W1 + b1) W2 + b2  -> hc rows 0..95
        with (
            tc.tile_pool(name="xt", bufs=1) as xtp,
            tc.tile_pool(name="e1", bufs=3) as e1p,
            tc.tile_pool(name="eps", bufs=2, space="PSUM") as epsp,
            tc.tile_pool(name="hps", bufs=2, space="PSUM") as hpsp,
        ):
            xt = xtp.tile([IN_DIM, N], f32)
            nc.sync.dma_start(xt[:], d_xT)
            for ti in range(NT):
                t0 = ti * T
                e_ps = epsp.tile([H, T], f32)
                MM(e_ps[:], s_ew1, xt[:, t0 : t0 + T], start=True, stop=True)
                e1 = e1p.tile([H, T], f32)
                nc.scalar.activation(e1[:], e_ps[:], AF.Silu, bias=s_eb1)
                h_ps = hpsp.tile([H, T], f32)
                MM(h_ps[:], s_ew2, e1[:], start=True, stop=True)
                nc.vector.tensor_scalar_add(
                    hc[0:H, PAD + t0 : PAD + t0 + T], h_ps[:], s_eb2
                )

        # ---- message-passing layers
        for l in range(L):
            wa, wb, w2 = s_wa[l], s_wb[l], s_w2[l]
            u1a, u1b, u2 = s_u1a[l], s_u1b[l], s_u2[l]
            b1, bupd, b2u = s_b1[l], s_bupd[l], s_b2u[l]

            # pass 1: messages + update MLP + residual + LN sums
            with (
                tc.tile_pool(name=f"pre{l}", bufs=2, space="PSUM") as prep,
                tc.tile_pool(name=f"agg{l}", bufs=2, space="PSUM") as aggp,
                tc.tile_pool(name=f"wrk{l}", bufs=1, space="PSUM") as wrkp,
                tc.tile_pool(name=f"md{l}", bufs=3) as mdp,
                tc.tile_pool(name=f"ss{l}", bufs=2) as ssp,
                tc.tile_pool(name=f"ag{l}", bufs=2) as agsp,
                tc.tile_pool(name=f"u{l}", bufs=2) as up,
                tc.tile_pool(name=f"zq{l}", bufs=2) as zqp,
            ):
                for ti in range(NT):
                    t0 = ti * T
                    c0 = PAD + t0
                    agg_ps = aggp.tile([H, T], f32)
                    for pi, (d0, d1) in enumerate(PAIRS):
                        pre = prep.tile([H, 2 * T], f32)
                        MM(pre[:, 0:T], wb, hc[:, c0 + d0 : c0 + d0 + T],
                           start=True, stop=False, skip_group_check=True)
                        MM(pre[:, T : 2 * T], wb, hc[:, c0 + d1 : c0 + d1 + T],
                           start=True, stop=False, skip_group_check=True)
                        MM(pre[:, 0:T], wa, hc[:, c0 : c0 + T],
                           start=False, stop=True, skip_group_check=True)
                        MM(pre[:, T : 2 * T], wa, hc[:, c0 : c0 + T],
                           start=False, stop=True, skip_group_check=True)
                        md = mdp.tile([H, 2 * T], f32)
                        nc.scalar.activation(md[:], pre[:], AF.Silu, bias=b1)
                        for half, d in ((0, d0), (1, d1)):
                            if ti == 0 and d < 0:
                                nc.gpsimd.memset(md[:, half * T : half * T - d], 0.0)
                            if ti == NT - 1 and d > 0:
                                nc.gpsimd.memset(
                                    md[:, (half + 1) * T - d : (half + 1) * T], 0.0
                                )
                        MM(agg_ps[:], w2, md[:, 0:T],
                           start=(pi == 0), stop=False, skip_group_check=True)
                        MM(agg_ps[:], w2, md[:, T : 2 * T],
                           start=False, stop=(pi == 3), skip_group_check=True)
                    aggc = agsp.tile([H, T], f32)
                    nc.vector.tensor_scalar_mul(aggc[:], agg_ps[:], 0.125)
                    if ti == 0:
                        nc.vector.tensor_mul(
                            aggc[:, 0:PAD], agg_ps[:, 0:PAD], s_invL
                        )
                    if ti == NT - 1:
                        nc.vector.tensor_mul(
                            aggc[:, T - PAD : T], agg_ps[:, T - PAD : T], s_invR
                        )
                    upd_ps = wrkp.tile([H, T], f32, tag="w")
                    MM(upd_ps[:], u1a, hc[0:H, c0 : c0 + T],
                       start=True, stop=False, skip_group_check=True)
                    MM(upd_ps[:], u1b, aggc[:],
                       start=False, stop=True, skip_group_check=True)
                    ut = up.tile([H, T], f32)
                    nc.scalar.activation(ut[:], upd_ps[:], AF.Silu, bias=bupd)
                    dh_ps = wrkp.tile([H, T], f32, tag="w")
                    MM(dh_ps[:], u2, ut[:], start=True, stop=True,
                       skip_group_check=True)
                    # z = (dh + b2u) + h
                    nc.vector.scalar_tensor_tensor(
                        zbuf[:, t0 : t0 + T], dh_ps[:], b2u,
                        hc[0:H, c0 : c0 + T], ALU.add, ALU.add,
                    )
                    zsq = zqp.tile([H, T], f32)
                    nc.vector.tensor_mul(
                        zsq[:], zbuf[:, t0 : t0 + T], zbuf[:, t0 : t0 + T]
                    )
                    st_ps = wrkp.tile([1, 2 * T], f32, tag="w")
                    MM(st_ps[:, 0:T], s_ones96, zbuf[:, t0 : t0 + T],
                       start=True, stop=True, skip_group_check=True)
                    MM(st_ps[:, T : 2 * T], s_ones96, zsq[:], start=True,
                       stop=True, skip_group_check=True)
                    sstat = ssp.tile([1, 2 * T], f32)
                    nc.vector.tensor_copy(sstat[:], st_ps[:])
                    nc.sync.dma_start(d_s1[0:1, t0 : t0 + T], sstat[:, 0:T])
                    nc.sync.dma_start(d_s2[0:1, t0 : t0 + T], sstat[:, T : 2 * T])

            # LN stats crunch on [128, 128] repack
            with tc.tile_pool(name=f"cr{l}", bufs=8) as crp:
                p1 = crp.tile([128, 128], f32)
                p2 = crp.tile([128, 128], f32)
                nc.sync.dma_start(p1[:], d_s1.rearrange("a (p c) -> (a p) c", p=128))
                nc.sync.dma_start(p2[:], d_s2.rearrange("a (p c) -> (a p) c", p=128))
                mu = crp.tile([128, 128], f32)
                nc.vector.tensor_scalar_mul(mu[:], p1[:], 1.0 / H)
                ex2 = crp.tile([128, 128], f32)
                nc.vector.tensor_scalar_mul(ex2[:], p2[:], 1.0 / H)
                var = crp.tile([128, 128], f32)
                # var = (ex2 + eps) - mu*mu
                nc.vector.tensor_mul(var[:], mu[:], mu[:])
                nc.vector.scalar_tensor_tensor(
                    var[:], ex2[:], LN_EPS, var[:], ALU.add, ALU.subtract
                )
                sd = crp.tile([128, 128], f32)
                nc.scalar.activation(sd[:], var[:], AF.Sqrt)
                isd = crp.tile([128, 128], f32)
                nc.vector.reciprocal(isd[:], sd[:])
                muis = crp.tile([128, 128], f32)
                nc.vector.tensor_mul(muis[:], mu[:], isd[:])
                nc.sync.dma_start(d_is.rearrange("a (p c) -> (a p) c", p=128), isd[:])
                nc.sync.dma_start(
                    d_muis.rearrange("a (p c) -> (a p) c", p=128), muis[:]
                )

            # pass 2: y = z*is - mu*is  -> hc rows 0..95
            with (
                tc.tile_pool(name=f"ou{l}", bufs=4, space="PSUM") as outp,
                tc.tile_pool(name=f"ir{l}", bufs=4) as irp,
                tc.tile_pool(name=f"tm{l}", bufs=2) as tmp_p,
            ):
                for ti in range(NT):
                    t0 = ti * T
                    isr = irp.tile([1, T], f32, tag="i")
                    nc.sync.dma_start(isr[:], d_is[0:1, t0 : t0 + T])
                    mur = irp.tile([1, T], f32, tag="m")
                    nc.sync.dma_start(mur[:], d_muis[0:1, t0 : t0 + T])
                    mis = outp.tile([H, T], f32)
                    MM(mis[:], s_onesr, isr[:], start=True, stop=True,
                       skip_group_check=True)
                    mmu = outp.tile([H, T], f32)
                    MM(mmu[:], s_onesr, mur[:], start=True, stop=True,
                       skip_group_check=True)
                    tmp = tmp_p.tile([H, T], f32)
                    nc.vector.tensor_mul(tmp[:], zbuf[:, t0 : t0 + T], mis[:])
                    nc.vector.tensor_sub(
                        hc[0:H, PAD + t0 : PAD + t0 + T], tmp[:], mmu[:]
                    )

        # ---- decode: out = dec_w2^T silu(dec_w1^T h + b1) + b2
        with (
            tc.tile_pool(name="dps", bufs=2, space="PSUM") as dpsp,
            tc.tile_pool(name="ops", bufs=2, space="PSUM") as opsp,
            tc.tile_pool(name="du", bufs=3) as dup,
            tc.tile_pool(name="ob", bufs=3) as obp,
        ):
            for ti in range(NT):
                t0 = ti * T
                d_ps = dpsp.tile([H, T], f32)
                MM(d_ps[:], s_dw1, hc[0:H, PAD + t0 : PAD + t0 + T],
                   start=True, stop=True, skip_group_check=True)
                du = dup.tile([H, T], f32)
                nc.scalar.activation(du[:], d_ps[:], AF.Silu, bias=s_db1)
                o_ps = opsp.tile([1, T], f32)
                MM(o_ps[:], s_dw2, du[:], start=True, stop=True,
                   skip_group_check=True)
                ob = obp.tile([1, T], f32)
                nc.vector.tensor_scalar_add(ob[:], o_ps[:], s_db2)
                nc.sync.dma_start(d_out[0:1, t0 : t0 + T], ob[:])

    _split_excess_waits(nc, mybir, maxw=1)
    return nc


def _get_nc():
    if "nc" not in _CACHE:
        _CACHE["nc"] = _build()
    return _CACHE["nc"]


def kernel(**inputs):
    from concourse.bass_utils import run_bass_kernel_spmd

    f = lambda a: np.ascontiguousarray(np.asarray(a, np.float32))
    x = f(inputs["x"])
    coord = f(inputs["coord"])
    msg_w1, msg_b1 = f(inputs["msg_w1"]), f(inputs["msg_b1"])
    msg_w2, msg_b2 = f(inputs["msg_w2"]), f(inputs["msg_b2"])
    upd_w1, upd_b1 = f(inputs["upd_w1"]), f(inputs["upd_b1"])
    upd_w2, upd_b2 = f(inputs["upd_w2"]), f(inputs["upd_b2"])
    ln_g, ln_b = f(inputs["ln_g"]), f(inputs["ln_b"])
    assert np.all(ln_g == 1.0) and np.all(ln_b == 0.0), "general LN affine unsupported"

    shared = {}
    for l in range(L):
        w1 = msg_w1[l]  # [2H+1, H]
        shared[f"wa{l}"] = np.concatenate([w1[:HID], -w1[2 * HID : 2 * HID + 1]], 0)
        shared[f"wb{l}"] = np.concatenate([w1[HID : 2 * HID], w1[2 * HID : 2 * HID + 1]], 0)
        shared[f"w2{l}"] = msg_w2[l]
        shared[f"u1a{l}"] = upd_w1[l][:HID]
        shared[f"u1b{l}"] = upd_w1[l][HID:]
        shared[f"u2{l}"] = upd_w2[l]
        shared[f"b1{l}"] = msg_b1[l].reshape(HID, 1)
        shared[f"bupd{l}"] = (upd_b1[l] + msg_b2[l] @ upd_w1[l][HID:]).reshape(HID, 1)
        shared[f"b2u{l}"] = upd_b2[l].reshape(HID, 1)
    shared["ew1"] = f(inputs["embed_w1"])
    shared["eb1"] = f(inputs["embed_b1"]).reshape(HID, 1)
    shared["ew2"] = f(inputs["embed_w2"])
    shared["eb2"] = f(inputs["embed_b2"]).reshape(HID, 1)
    shared["dw1"] = f(inputs["dec_w1"])
    shared["db1"] = f(inputs["dec_b1"]).reshape(HID, 1)
    shared["dw2"] = f(inputs["dec_w2"]).reshape(HID, 1)
    shared["db2"] = f(inputs["dec_b2"]).reshape(1, 1)
    shared["ones96"] = np.ones((HID, 1), np.float32)
    shared["onesr"] = np.ones((1, HID), np.float32)
    idx = np.arange(N, dtype=np.float64)
    count = np.maximum(np.minimum(idx, K) + np.minimum(N - 1 - idx, K), 1.0)
    shared["invL"] = np.broadcast_to(
        (1.0 / count[:PAD]).astype(np.float32), (HID, PAD)
    ).copy()
    shared["invR"] = np.broadcast_to(
        (1.0 / count[-PAD:]).astype(np.float32), (HID, PAD)
    ).copy()
    shared = {k: f(v) for k, v in shared.items()}

    in_maps = []
    for c in range(B):
        m = dict(shared)
        m["xT"] = np.ascontiguousarray(x[c].T)
        cp = np.zeros((1, NP), np.float32)
        cp[0, PAD : PAD + N] = coord[c]
        m["coordp"] = cp
        in_maps.append(m)

    nc = _get_nc()
    res = run_bass_kernel_spmd(
        nc, in_maps, core_ids=list(range(B)), trace=bool(_CACHE.get("trace"))
    )
    _CACHE["last_res"] = res
    out = np.stack([res.results[c]["out"].reshape(N, 1) for c in range(B)])
    return out

